# revision 5
# baseline (speedup 1.0000x reference)
"""Trainium2 (Bass/Tile) segment-sum kernel, 8-core SPMD — v3 (lid-groups).

Computes out[v, :] = sum over rows n with X_node[n] == v of H[n, :]
(= jax.ops.segment_sum(H, X_node, num_segments=V)).

v3 structure (changes vs v2 in *bold*):
  host: stable-argsort rows by segment; 8 contiguous core chunks; greedy
    windows of <=128-segment span. *Within a window, each segment's rows
    are padded to a multiple of G and packed into "groups": G consecutive
    tile-columns at one partition that all hold rows of the SAME segment.*
    (~1.6% zero-row padding for G=2, ~4.7% for G=4.)
  device, per (window w, group q):
    - ONE DVE tensor_scalar builds the staircase u[p, v] = (iota[v] >=
      lid[p, q]) for ALL G columns of the group at once (they share lid).
    - ONE matmul with moving operand [128, G*128] (the group's G row-
      tiles, contiguous in SBUF) and a *stride-0 PSUM out AP* accumulates
      all G tiles into acc[v, :]: PSUM's has_written bit makes within-
      instruction revisits of the same element accumulate. One hidden
      LDWEIGHTS per G tiles keeps TensorE dense (HAM stays warm).
    PSUM strips hold cumulative-along-v segment sums as in v2.
  host: seg sums = diff of strips along v (prepend 0); add into out.

vs v2 measured (215.6us): DVE builds 1189x163ns=193us -> /G; PE
1566 MM + 1566 LDW (177+131us) -> T/G wide MMs, loads hidden. New
bound: DMA ~150-160us (51.2MB bf16 H * (1+pad) + 3.5MB f32 strips).
"""

import os

import numpy as np
from contextlib import ExitStack

import ml_dtypes
import concourse.bass as bass
import concourse.tile as tile
from concourse import bacc, mybir
from concourse.bass_utils import run_bass_kernel_spmd

F32 = mybir.dt.float32
BF16 = mybir.dt.bfloat16
NP_BF16 = ml_dtypes.bfloat16
P = 128  # partitions / tile rows / max window width (segments)
D = 128  # feature dim
N_CORES = 8
PAD_LID = 255.0

LAST_RESULTS = None  # test-harness hook: BassKernelResults of the last run
_NC_CACHE = {}  # (W, T, G) -> compiled Bacc program


def _t_candidates(G: int):
    # T must be a multiple of G (whole groups) and the half-split point
    # G*ceil(T/(2G)) stays G-aligned automatically.
    return tuple(t for t in (24, 26, 28, 30, 32) if t % G == 0)


def _build_nc_cached(W: int, T: int, G: int):
    key = (W, T, G)
    if key not in _NC_CACHE:
        _NC_CACHE[key] = _build_nc(W, T, G)
    return _NC_CACHE[key]


def _build_nc(W: int, T: int, G: int):
    Q = T // G  # groups per window
    nc = bacc.Bacc(
        "TRN2",
        target_bir_lowering=False,
        debug=False,
        enable_asserts=False,
        num_devices=N_CORES,
    )
    # h[w, p, (t, d)] -- per-partition contiguous runs of T*D*2B
    h = nc.dram_tensor("h", [W, P, T * D], BF16, kind="ExternalInput")
    lid = nc.dram_tensor("lid", [P, W * Q], BF16, kind="ExternalInput")
    iota = nc.dram_tensor("iota", [P, P], BF16, kind="ExternalInput")
    bdiag = nc.dram_tensor("bdiag", [P, P], F32, kind="ExternalInput")
    # [P, W*D]: strips of 4 windows batch into one >=1KB-per-partition
    # store (256B runs sit below the SDMA 512B line-rate knee)
    out = nc.dram_tensor("out", [P, W * D], BF16, kind="ExternalOutput")

    with tile.TileContext(nc) as tc, ExitStack() as ctx:
        const = ctx.enter_context(tc.tile_pool(name="const", bufs=1))
        hpool = ctx.enter_context(tc.tile_pool(name="hw", bufs=40))
        ohpool = ctx.enter_context(tc.tile_pool(name="oh", bufs=20))
        mspool = ctx.enter_context(tc.tile_pool(name="ms", bufs=8))
        opool = ctx.enter_context(tc.tile_pool(name="ot", bufs=16))
        psum = ctx.enter_context(tc.tile_pool(name="acc", bufs=6, space="PSUM"))
        psum2 = ctx.enter_context(tc.tile_pool(name="acc2", bufs=2, space="PSUM"))

        qh = (Q + 1) // 2  # groups in the first half-load
        halves = [(0, qh), (qh, Q)]  # in group units

        def load_h(w, q0, q1):
            ht = hpool.tile([P, (q1 - q0) * G * D], BF16, tag="ht")
            nc.sync.dma_start(ht[:], h[w][:, q0 * G * D : q1 * G * D])
            return ht

        hts = {}
        for w in range(min(3, W)):
            for q0, q1 in halves:
                hts[(w, q0)] = load_h(w, q0, q1)

        # constants go via the ACT ring so they land immediately instead of
        # queueing behind the hoisted bulk loads on the SP ring
        iota_sb = const.tile([P, P], BF16)
        nc.scalar.dma_start(iota_sb[:], iota[:])
        # lid ships as bf16 (exact for <=255) and is cast once to the f32
        # scratch that tensor_scalar's comparison scalar requires
        lid_bf = const.tile([P, W * Q], BF16)
        nc.scalar.dma_start(lid_bf[:], lid[:])
        lid_sb = const.tile([P, W * Q], F32)
        nc.scalar.copy(lid_sb[:], lid_bf[:])
        bdiag_sb = const.tile([P, P], F32)
        nc.scalar.dma_start(bdiag_sb[:], bdiag[:])

        SG = 4  # windows per store group
        ot = None
        for w in range(W):
            acc = psum.tile([P, D], F32)
            for q0, q1 in halves:
                if (w, q0) in hts:
                    ht = hts.pop((w, q0))
                else:
                    ht = load_h(w, q0, q1)
                for q in range(q0, q1):
                    # u[p, v] = (iota[v] >= lid[p, w*Q + q]); shared by the
                    # group's G tile-columns
                    oh = ohpool.tile([P, P], BF16)
                    nc.vector.tensor_scalar(
                        oh[:],
                        iota_sb[:],
                        lid_sb[:, w * Q + q : w * Q + q + 1],
                        None,
                        mybir.AluOpType.is_ge,
                    )
                    # one matmul streams the group's G tiles; the stride-0
                    # out AP revisits acc[v, :] G times -> PSUM accumulates
                    nc.tensor.matmul(
                        acc[:].unsqueeze(1).broadcast_to((P, G, D)),
                        oh[:],
                        ht[:, (q - q0) * G * D : (q - q0 + 1) * G * D],
                        start=(q == 0),
                        stop=(q == Q - 1),
                    )
            # diff along v on-device: one fp32 matmul with the fixed
            # bidiagonal B (B[p,p]=1, B[p,p+1]=-1) turns the cumulative
            # strip into per-segment sums exactly (+-1 multiplies), so the
            # now-small values ship as bf16 (halves the output DMA)
            ms = mspool.tile([P, D], F32)
            nc.scalar.copy(ms[:], acc[:])
            acc2 = psum2.tile([P, D], F32)
            nc.tensor.matmul(acc2[:], bdiag_sb[:], ms[:], start=True, stop=True)
            g0 = (w // SG) * SG
            gl = min(SG, W - g0)
            if w == g0:
                ot = opool.tile([P, gl * D], BF16, tag="ot")
            nc.scalar.copy(ot[:, (w - g0) * D : (w - g0 + 1) * D], acc2[:])
            if w == g0 + gl - 1:
                nc.scalar.dma_start(out[:, g0 * D : (g0 + gl) * D], ot[:])

    nc.compile()
    return nc


def _prepare(H: np.ndarray, X: np.ndarray, V: int, G: int):
    """Host-side sort + greedy grouping + bf16 cast + swizzle.

    Returns (in_maps, wbase[k, w], W, T).
    """
    N, Dd = H.shape
    assert Dd == D and N % N_CORES == 0
    nloc = N // N_CORES
    X = np.ascontiguousarray(X).astype(np.int64, copy=False)
    perm = np.argsort(X, kind="stable")
    sidx = X[perm]

    # Per-core segment runs (pieces) in sorted order.
    def runs_for_core(k):
        s = sidx[k * nloc : (k + 1) * nloc]
        segs, counts = np.unique(s, return_counts=True)
        return segs, counts

    core_runs = [runs_for_core(k) for k in range(N_CORES)]

    def greedy(T):
        """Pack segments (splitting across windows allowed) into windows of
        <=128-seg span and <=128*(T/G) groups. Returns per-core window list:
        (wbase, [(seg, take_rows), ...]) and W."""
        Q = T // G
        capq = P * Q  # groups per window
        allw = []
        for k in range(N_CORES):
            segs, counts = core_runs[k]
            wins = []
            i, rem = 0, 0  # segment index, rows already consumed of segs[i]
            while i < len(segs):
                base = segs[i]
                used = 0
                pieces = []
                while i < len(segs) and segs[i] < base + P:
                    r = counts[i] - rem
                    g = (r + G - 1) // G
                    if used + g <= capq:
                        pieces.append((segs[i], r))
                        used += g
                        rem = 0
                        i += 1
                    else:
                        take = (capq - used) * G
                        if take > 0:
                            pieces.append((segs[i], take))
                            rem += take
                        used = capq
                        break
                wins.append((base, pieces))
            allw.append(wins)
        W = max(len(w) for w in allw)
        return allw, W

    best = None
    for T in _t_candidates(G):
        allw, W = greedy(T)
        if best is None or W * T < best[2] * best[1]:
            best = (allw, T, W)
    allw, T, W = best
    Q = T // G
    cap = T * P

    # Build the swizzled H, the per-group lid matrix, and wbase.
    total = N_CORES * W * cap
    hi = H.astype(NP_BF16)
    Hp = np.zeros((total, D), NP_BF16)
    lid_q = np.full((N_CORES, W, P, Q), PAD_LID, np.float32)
    wbase = np.full((N_CORES, W), V, np.int64)

    for k in range(N_CORES):
        row = k * nloc  # next sorted-row index to place
        for w, (base, pieces) in enumerate(allw[k]):
            wbase[k, w] = base
            j = 0  # group index within window
            for seg, r in pieces:
                g = (r + G - 1) // G
                jj = j + np.arange(g)
                lid_q[k, w, jj % P, jj // P] = seg - base
                # rows of this piece -> slots
                idx = np.arange(r)
                gj = j + idx // G  # global group idx
                p_ = gj % P
                t_ = (gj // P) * G + (idx % G)
                slot = ((k * W + w) * P + p_) * T + t_
                Hp[slot] = hi[perm[row : row + r]]
                row += r
                j += g
        assert row == (k + 1) * nloc

    Hp = Hp.reshape(N_CORES, W, P, T * D)
    lid_mat = np.ascontiguousarray(
        lid_q.transpose(0, 2, 1, 3).reshape(N_CORES, P, W * Q).astype(NP_BF16)
    )

    iota = np.ascontiguousarray(
        np.broadcast_to(np.arange(P, dtype=np.float32).astype(NP_BF16), (P, P))
    )
    bdiag = np.ascontiguousarray(
        np.eye(P, dtype=np.float32) - np.eye(P, k=1, dtype=np.float32)
    )

    in_maps = [
        {"h": Hp[k], "lid": lid_mat[k], "iota": iota, "bdiag": bdiag}
        for k in range(N_CORES)
    ]
    return in_maps, wbase, W, T


def kernel(H, X_node, V, trace: bool = False) -> np.ndarray:
    global LAST_RESULTS
    H = np.asarray(H, dtype=np.float32)
    X = np.asarray(X_node)
    V = int(V)

    G = int(os.environ.get("SEGSUM_G", "2"))
    in_maps, wbase, W, T = _prepare(H, X, V, G)
    nc = _build_nc_cached(W, T, G)
    res = run_bass_kernel_spmd(nc, in_maps, list(range(N_CORES)), trace=trace)
    LAST_RESULTS = res

    out = np.zeros((V + P, D), np.float32)
    for k in range(N_CORES):
        # strips are already diffed on-device: per-segment sums in bf16,
        # laid out [P, W, D]
        d = np.asarray(res.results[k]["out"]).astype(np.float32)
        d = d.reshape(P, W, D)
        for w in range(W):
            b = int(wbase[k, w])
            out[b : b + P] += d[:, w, :]
    return np.ascontiguousarray(out[:V])
